# revision 29
# baseline (speedup 1.0000x reference)
"""SpMM (COO adjacency @ dense weight) on 8 Trainium2 NeuronCores.

out[r] = sum over edges (r, c) of weight[c]   (adj values are all ones)

Strategy: partition edges by destination row across the 8 cores (see
sharding hint). Host packs output rows into 8*T bins of <=128 rows AND
<=128 incoming edges each (capacity-aware best-fit over T=100 tiles/core,
which leaves ~2.4% slot slack so exact packing succeeds). Per core the
host builds a compacted bf16 weight table holding only that core's
unique source columns (~11.8K rows, so slot indices fit int16) plus an
int16 slot->table-row index list and a per-slot local-dest array.

Device work per core (all bf16 data path, fp32 PSUM accumulate):
  1. dma_gather (MoE-style SWDGE gather ucode) pulls CHUNK*128 table
     rows per instruction into SBUF as [128, CHUNK, 256] — slot i lands
     at partition i%128, free slot i//128, exactly the matmul rhs
     layout. ~10 gather instructions replace the baseline's 98
     serialized indirect DMAs (descriptor-gen cost 994ns + 0.34ns/row).
  2. Per tile, Vector builds the 0/1 selection matrix
     S[e, r] = (dest[e] == r) in bf16; TensorEngine matmul
     psum[r, :] += S^T @ gathered does the segment-sum (bf16 matmul is
     4x the fp32 rate).
  3. PSUM -> SBUF bf16 copies (alternating Vector/Scalar engines),
     staged CHUNK tiles at a time, then one 640KB HWDGE write per chunk
     to a partition-major [128, T, 256] bf16 output tensor.
Host inverse-permutes the per-core outputs and upcasts to f32 (bf16
rounding of in/out is ~0.4% worst case vs the 2e-2 tolerance).
"""

import heapq
import os

import ml_dtypes
import numpy as np

NC_CORES = 8
P = 128
T_TILES = 100  # output tiles (bins) per core; 8*T*128 row slots total
CHUNK = 10  # tiles per input-stream DMA (or dma_gather) / per output write
# "gather": device-side dma_gather from a per-core unique-column table.
# "stream": host lays per-slot rows out in gather-result order; device
#           streams them with bulk HWDGE DMAs (no SWDGE desc-gen).
MODE = os.environ.get("KMODE", "gather")


def _build_program(n_tbl, d, t_tiles, chunk, mode=None):
    """Build the SPMD Bass program (identical across cores; data differs)."""
    from contextlib import ExitStack

    import concourse.bacc as bacc
    import concourse.mybir as mybir
    import concourse.tile as tile

    mode = mode or MODE
    dt = mybir.dt
    nc = bacc.Bacc(None, num_swdge_queues=4)

    idx_cols = (t_tiles * P) // 16
    if mode == "gather":
        wt = nc.declare_dram_parameter("wt", [n_tbl, d], dt.bfloat16, isOutput=False)
        idx_p = nc.declare_dram_parameter(
            "idx", [P, idx_cols], dt.int16, isOutput=False
        )
    else:
        wt = nc.declare_dram_parameter(
            "wt", [P, t_tiles, d], dt.bfloat16, isOutput=False
        )
    dest_p = nc.declare_dram_parameter(
        "dest", [P, t_tiles], dt.bfloat16, isOutput=False
    )
    iota_p = nc.declare_dram_parameter("iota", [P, P], dt.bfloat16, isOutput=False)
    out_p = nc.declare_dram_parameter(
        "out", [P, t_tiles, d], dt.bfloat16, isOutput=True
    )

    with tile.TileContext(nc) as tc:
        with ExitStack() as ctx:
            n_chunks = -(-t_tiles // chunk)
            cpool = ctx.enter_context(tc.tile_pool(name="const", bufs=1))
            # one buffer per chunk: stream-in and staging never recycle,
            # so the input stream can run arbitrarily far ahead
            gpool = ctx.enter_context(tc.tile_pool(name="g", bufs=n_chunks))
            spool = ctx.enter_context(tc.tile_pool(name="s", bufs=n_chunks))
            opool = ctx.enter_context(tc.tile_pool(name="o", bufs=n_chunks))
            pspool = ctx.enter_context(tc.tile_pool(name="ps", bufs=4, space="PSUM"))

            if mode == "gather":
                idx_sb = cpool.tile([P, idx_cols], dtype=dt.int16)
                nc.sync.dma_start(idx_sb[:], idx_p[:])
            iota_sb = cpool.tile([P, P], dtype=dt.bfloat16)
            nc.sync.dma_start(iota_sb[:], iota_p[:])
            dest_sb = cpool.tile([P, t_tiles], dtype=dt.bfloat16)
            nc.sync.dma_start(dest_sb[:], dest_p[:])

            chunks = []
            # prologue: issue every input-stream chunk (self-driven pipeline;
            # per-chunk buffers never recycle so nothing blocks issue)
            for gi, g0 in enumerate(range(0, t_tiles, chunk)):
                k = min(chunk, t_tiles - g0)
                gt = gpool.tile([P, k, d], dtype=dt.bfloat16, tag="g")
                if mode == "gather":
                    nc.gpsimd.dma_gather(
                        out_ap=gt[:],
                        in_ap=wt[:],
                        idxs_ap=idx_sb[:, g0 * 8 : (g0 + k) * 8],
                        num_idxs=P * k,
                        num_idxs_reg=P * k,
                        elem_size=d,
                        # >64 descriptors per SDMA engine don't fit one
                        # packet; the packed-packet path DMA-aborts >1024.
                        single_packet=False,
                        # queue 0 desc-gen occupies the Pool engine; 1-3
                        # run concurrently off-engine. Dispatch async queues
                        # first so q0's on-engine gen doesn't delay them.
                        queue_num=(1, 2, 3, 0)[gi % 4],
                    )
                else:
                    # Activation-issued HWDGE: keeps the input stream off the
                    # SP ring so it never queues behind a blocked out-write.
                    nc.scalar.dma_start(gt[:], wt[:, g0 : g0 + k, :])
                # selection matrices for the whole chunk in one DVE op
                s = spool.tile([P, k, P], dtype=dt.bfloat16, tag="s")
                nc.vector.tensor_tensor(
                    out=s[:],
                    in0=dest_sb[:, g0 : g0 + k].unsqueeze(2).to_broadcast([P, k, P]),
                    in1=iota_sb[:].unsqueeze(1).to_broadcast([P, k, P]),
                    op=mybir.AluOpType.is_equal,
                )
                chunks.append((g0, k, gt, s))

            ci = 0
            for g0, k, gt, s in chunks:
                ot = opool.tile([P, k, d], dtype=dt.bfloat16, tag="o")
                # matmuls in groups of 4 sharing one 2-bank PSUM tile; one
                # big cast-copy per group (fixed DVE/ACT op cost amortizes
                # over FD=1024), alternating Scalar/Vector (GPSIMD cannot
                # read PSUM; ScalarE is the faster PSUM reader at large FD)
                for j0 in range(0, k, 4):
                    m = min(4, k - j0)
                    ps = pspool.tile([P, 4, d], dtype=dt.float32)
                    for j in range(m):
                        nc.tensor.matmul(
                            out=ps[:, j, :],
                            lhsT=s[:, j0 + j, :],
                            rhs=gt[:, j0 + j, :],
                            start=True,
                            stop=True,
                        )
                    if ci % 2 == 0:
                        nc.scalar.copy(out=ot[:, j0 : j0 + m, :], in_=ps[:, :m, :])
                    else:
                        nc.vector.tensor_copy(
                            out=ot[:, j0 : j0 + m, :], in_=ps[:, :m, :]
                        )
                    ci += 1
                nc.sync.dma_start(out_p[:, g0 : g0 + k, :], ot[:])

    nc.finalize()
    return nc


def _pack_bins_exact(rows, counts, nbins):
    """Best-fit pack rows into bins with <=128 slots AND <=128 rows each.

    Returns (bin_of_row, pos_of_row) or None if infeasible.
    """
    n = len(counts)
    if nbins * P < counts.sum() or counts.max() > P:
        return None
    nz = np.flatnonzero(counts)
    order = nz[np.argsort(-counts[nz], kind="stable")]
    bin_of_row = np.full(n, -1, np.int64)
    loads = np.zeros(nbins, np.int64)
    nrows = np.zeros(nbins, np.int64)
    heap = [(0, b) for b in range(nbins)]
    heapq.heapify(heap)
    for r in order.tolist():
        c = int(counts[r])
        while True:
            if not heap:
                return None
            load, b = heapq.heappop(heap)
            if load != loads[b] or nrows[b] >= P:
                continue  # stale entry or row-capacity full
            break
        if load + c > P:
            return None  # min-load bin can't fit -> nothing can
        bin_of_row[r] = b
        loads[b] += c
        nrows[b] += 1
        if loads[b] < P and nrows[b] < P:
            heapq.heappush(heap, (int(loads[b]), b))
    # zero-count rows fill the remaining row capacity anywhere
    zeros = np.flatnonzero(counts == 0)
    cap = P - nrows
    if cap.sum() < len(zeros):
        return None
    fill_bins = np.repeat(np.arange(nbins), cap)[: len(zeros)]
    bin_of_row[zeros] = fill_bins
    # positions: stable order within bin
    order_all = np.argsort(bin_of_row, kind="stable")
    bins_sorted = bin_of_row[order_all]
    starts = np.searchsorted(bins_sorted, np.arange(nbins))
    pos_of_row = np.empty(n, np.int64)
    pos_of_row[order_all] = np.arange(n, dtype=np.int64) - starts[bins_sorted]
    if pos_of_row.max() >= P:
        return None
    return bin_of_row, pos_of_row


def _prepare(adj, weight):
    """Host-side sharding: pack rows into bins, build per-core gather data."""
    w = np.ascontiguousarray(np.asarray(weight, dtype=np.float32))
    n, d = w.shape
    adj = np.asarray(adj)
    rows = adj[0].astype(np.int64)
    cols = adj[1].astype(np.int64)

    counts = np.bincount(rows, minlength=n)
    t_tiles = T_TILES
    while True:
        nbins = NC_CORES * t_tiles
        packed = _pack_bins_exact(rows, counts, nbins)
        if packed is not None:
            break
        t_tiles += 2  # more slack; terminates long before degree bound bites
    bin_of_row, pos_of_row = packed

    # Edge slots: edges of a bin occupy consecutive slots ordered by source
    # column (ascending table reads within each tile chunk).
    eb = bin_of_row[rows]
    eo = np.lexsort((cols, eb))
    sb = eb[eo]
    starts = np.searchsorted(sb, np.arange(nbins))
    slot_in_bin = np.arange(len(eo), dtype=np.int64) - starts[sb]

    w_bf = w.astype(ml_dtypes.bfloat16)
    slots = t_tiles * P

    per_core = []
    for c in range(NC_CORES):
        sel = (sb // t_tiles) == c
        cols_c = cols[eo[sel]]
        gslot = (sb[sel] % t_tiles) * P + slot_in_bin[sel]
        uniq, inv = np.unique(cols_c, return_inverse=True)
        assert len(uniq) < 32768, len(uniq)
        idx_flat = np.zeros(slots, np.int16)
        dest_flat = np.full(slots, -1.0, np.float32)
        idx_flat[gslot] = inv.astype(np.int16)
        dest_flat[gslot] = pos_of_row[rows[eo[sel]]].astype(np.float32)
        col_flat = np.zeros(slots, np.int64)
        col_flat[gslot] = cols_c
        per_core.append((uniq, idx_flat, dest_flat, col_flat))

    n_tbl = max(len(u) for u, _, _, _ in per_core)
    n_tbl = -P * (-n_tbl // P)  # round up to multiple of 128

    iota = np.ascontiguousarray(
        np.broadcast_to(np.arange(P).astype(ml_dtypes.bfloat16), (P, P))
    )
    in_maps = []
    for uniq, idx_flat, dest_flat, col_flat in per_core:
        if MODE == "gather":
            tbl = np.zeros((n_tbl, d), ml_dtypes.bfloat16)
            tbl[: len(uniq)] = w_bf[uniq]
        else:
            # slot-ordered rows, partition-major: tbl[p, t, :] = row of
            # slot t*128+p (the layout dma_gather would produce).
            tbl = np.ascontiguousarray(
                w_bf[col_flat].reshape(t_tiles, P, d).transpose(1, 0, 2)
            )
        idx_wrapped = np.ascontiguousarray(idx_flat.reshape(-1, 16).T)  # [16, slots/16]
        idx_full = np.ascontiguousarray(np.tile(idx_wrapped, (8, 1)))  # [128, ...]
        dest_arr = np.ascontiguousarray(
            dest_flat.reshape(t_tiles, P).T.astype(ml_dtypes.bfloat16)
        )  # [128, T]
        m = {"wt": tbl, "dest": dest_arr, "iota": iota}
        if MODE == "gather":
            m["idx"] = idx_full
        in_maps.append(m)

    meta = {
        "n": n,
        "d": d,
        "t_tiles": t_tiles,
        "n_tbl": n_tbl,
        "bin_of_row": bin_of_row,
        "pos_of_row": pos_of_row,
    }
    return in_maps, meta


LAST_RESULT = None


def kernel(adj, size, weight):
    global LAST_RESULT
    from concourse.bass_utils import run_bass_kernel_spmd

    in_maps, meta = _prepare(adj, weight)
    nc = _build_program(meta["n_tbl"], meta["d"], meta["t_tiles"], CHUNK)
    res = run_bass_kernel_spmd(nc, in_maps, core_ids=list(range(NC_CORES)))
    LAST_RESULT = res
    t_tiles = meta["t_tiles"]
    # stack: [core, 128, T, d] -> index rows by (core, pos, local_tile)
    big = np.stack([np.asarray(r["out"]) for r in res.results])
    b = meta["bin_of_row"]
    out = big[b // t_tiles, meta["pos_of_row"], b % t_tiles, :]
    return np.ascontiguousarray(out.astype(np.float32))
